# revision 33
# baseline (speedup 1.0000x reference)
"""Trainium2 Bass kernel for nn_MessageLayer (GNN message passing), 8 NeuronCores.

Reference computation:
    edge_mat = (edge_features @ W + b).reshape(E, 64, 16)
    messages = einsum('emh,eh->em', edge_mat, hidden[edge_sources])
    out      = segment_sum(messages, edge_targets, num_segments=10000)

Algebraic restructure (cuts FLOPs 32x): since aggregation is linear,
    out[n, m] = sum_{f,h} W[f, m*16+h] * C[n, f, h],
    C[n, f, h] = sum_{e: tgt(e)=n} ef[e, f] * hidden[src(e), h]

Per-target segments ("positions", split at 64 edges) pack into K=128
matmul tiles in two species: BIG (33..64 edges, 2 row-slots of 64) and
SMALL (<=32, 4 row-slots of 32), x 4 ef column-classes each.

C stage: each tile runs as 4 CONCURRENT col-group matmuls (one per ef
class, tile_position=(0,32g)), so class g writes only PSUM partitions
[32g,32g+32) and the four classes stack into the same PSUM columns at
different partition bands: every drained column is fully valid (4x less
PSUM/drain traffic than a fused [128,N] matmul).  Whole-double-bank
[128,1024] f32->bf16 drains alternate DVE/ACT into c_spread.

W stage: c_spread column (t,j,h) holds classes 0-3 stacked, so two
block-diagonal K=64 strips (classes {0,1} at partitions 0-64 into poA,
{2,3} at 64-128 into poB, tile_position=(64s,0)) each stream [64,U]
stride-16 over 16 accumulating h-phases.  Issued piecewise: the big-u
piece right after the big species, then one piece per small double-bank
as soon as it drains, so only the last double-bank's W work (48 u) runs
as post-DMA tail; pad u-slots are skipped (po pads memset once).

Data movement: stationary ef tiles ride bf16; the moving operand ships
as a FULL-HEIGHT band-major fp8-e3m4 image (mixed-dtype matmul), so all
slot-separation zeros come from the host and every DMA is
dependency-free -- no on-device memsets, no ring reuse, nothing the
Tile scheduler can head-block.  St chunks and mv chunks pair across the
two HWDGE queues (sync/scalar); w5 rides the gpsimd SWDGE queue.
90 full-row warmup matmuls engage the HAM clock gate before the C stage
(1-row or 32-row-strip matmuls do not register as PE-busy).

Sharding: node-ownership (scatter-reduce by target): core c owns nodes
[1250c, 1250c+1250) and receives exactly the edges targeting them, so no
collective is needed; host assembles per-position rows into the output.
Numerics: hidden in e3m4 + bf16 elsewhere + bf16 drains/output:
rel-err ~1.2e-2 vs the 2e-2 gate (measured on the seeded inputs).
"""
import numpy as np
from contextlib import ExitStack

N_NODES = 10000
N_EDGES = 320000
HID = 16
MSG = 64
EFD = 32
NCORES = 8
NPC = N_NODES // NCORES          # 1250 nodes owned per core
CPBUFS = 4                       # PSUM tiles for C banks (4 + 4 po = 8)

_CACHE = {}


def _bf16():
    import ml_dtypes
    return ml_dtypes.bfloat16


def _f8e3():
    import ml_dtypes
    return ml_dtypes.float8_e3m4


def _build_layout(edge_targets):
    """Per-core position lists (node, edge-ids, len<=64, sorted desc; all
    len>32 "big" positions precede the "small" ones) plus the SPMD-uniform
    grid: T_big 8-position matmuls then T_small 16-position matmuls."""
    segs_per_core, nbig_per_core = [], []
    for c in range(NCORES):
        lo = c * NPC
        mask = (edge_targets >= lo) & (edge_targets < lo + NPC)
        eids = np.nonzero(mask)[0]
        tgt = edge_targets[eids]
        order = np.argsort(tgt, kind="stable")
        eids = eids[order]
        tgt = tgt[order]
        segs = []
        uniq, starts = np.unique(tgt, return_index=True)
        bounds = list(starts) + [len(tgt)]
        for i, n in enumerate(uniq):
            s, e = bounds[i], bounds[i + 1]
            while e - s > 64:
                segs.append((int(n), eids[s:s + 64]))
                s += 64
            segs.append((int(n), eids[s:e]))
        segs.sort(key=lambda t: -len(t[1]))
        segs_per_core.append(segs)
        nbig_per_core.append(sum(1 for _, e in segs if len(e) > 32))

    T_big = -(-max(nbig_per_core) // 8)
    T_big = ((T_big + 7) // 8) * 8            # whole double-banks of 8 matmuls
    nsmall = max(len(s) - b for s, b in zip(segs_per_core, nbig_per_core))
    T_small = -(-nsmall // 16)
    T_small = ((T_small + 3) // 4) * 4        # whole double-banks of 4 matmuls
    # U = po/out column count: 64 u-slots per drained double-bank
    # (big: u = 2t+j, 32 tiles/double-bank; small: u = DB_big*64+4t+j)
    DB_big = -(-T_big // 32)
    DB_small = -(-T_small // 16)
    U = (DB_big + DB_small) * 64
    assert U <= 512, f"U={U} exceeds one PSUM bank"
    return segs_per_core, nbig_per_core, T_big, T_small, U


def _build_w2(W):
    # w5[32g+f, 128h + 64*(g%2) + m] = W[f, m*16+h]: block-diagonal pairs
    # so the W-stage contracts K=64 (classes g, g+1 stacked per column)
    Wr = W.reshape(EFD, MSG, HID)                      # [f, m, h]
    w5 = np.zeros((128, 16 * 128), dtype=np.float32)
    for g in range(4):
        a = g % 2
        for h in range(HID):
            w5[32 * g:32 * g + 32, 128 * h + 64 * a:128 * h + 64 * a + 64] \
                = Wr[:, :, h]
    # w7: two zero-padded copies so each W matmul contracts the FULL
    # K=128 partition range (zero rows kill the other class pair) --
    # full-row matmuls register as PE-busy for the HAM clock gate,
    # so the W stage itself holds the PE at 2.4 GHz.
    w7 = np.concatenate([w5 * (np.arange(128) < 64)[:, None],
                         w5 * (np.arange(128) >= 64)[:, None]], axis=1)
    return w7


def _pack_core(segs, nbig, T_big, T_small, w2, edge_features, edge_sources,
               hidden):
    """DRAM image per core, bf16:
      [128, T*128 st | T_big*64 mv-big | T_small*64 mv-small | 2048 w5]
    BIG position p<8*T_big (t=p//8, j=(p//4)%2, g=p%4):
      st[64j+r, t*128+32g+f];  mv-big band j at partitions 64j: [64j+r, t*64+16g+h]
    SMALL position q (t=q//16, j=(q//4)%4, g=q%4):
      st[32j+r, (T_big+t)*128+32g+f];  mv-small band j at partitions 32j."""
    T = T_big + T_small
    St = np.zeros((128, T * 128), dtype=np.float32)
    # full-height band-major mv image in fp8 e3m4: slot-separation zeros
    # come from the host for free (fp8 halves the bytes, so full-height
    # costs about the same wire traffic as 2-byte compact bands), which
    # makes every DMA dependency-free -- no on-device memsets, no ring.
    Mv = np.zeros((128, (2 * T_big + 4 * T_small) * 64), dtype=np.float32)
    MVS0 = 2 * T_big * 64
    for i in range(len(segs)):
        _, eids = segs[i]
        k = len(eids)
        if i < nbig:
            t, j, g = i // 8, (i // 4) % 2, i % 4
            r0 = 64 * j
            St[r0:r0 + k, t * 128 + 32 * g:t * 128 + 32 * g + EFD] = \
                edge_features[eids]
            c0 = j * T_big * 64 + t * 64 + 16 * g
            Mv[r0:r0 + k, c0:c0 + HID] = hidden[edge_sources[eids]]
        else:
            q = i - nbig
            t, j, g = q // 16, (q // 4) % 4, q % 4
            r0 = 32 * j
            St[r0:r0 + k, (T_big + t) * 128 + 32 * g:
               (T_big + t) * 128 + 32 * g + EFD] = edge_features[eids]
            c0 = MVS0 + j * T_small * 64 + t * 64 + 16 * g
            Mv[r0:r0 + k, c0:c0 + HID] = hidden[edge_sources[eids]]
    D = np.concatenate([St, w2], axis=1)
    return (np.ascontiguousarray(D.astype(_bf16())),
            np.ascontiguousarray(Mv.astype(_f8e3())))


def _chunks(T, n, align):
    bs = [((round(k * T / n)) // align) * align for k in range(n)] + [T]
    bs[1] = max(bs[1], align) if T >= align else bs[1]
    return [(bs[k], bs[k + 1]) for k in range(n) if bs[k + 1] > bs[k]]


def _build_program(T_big, T_small, U):
    import concourse.tile as tile
    from concourse import bacc, mybir

    f32 = mybir.dt.float32
    bf16 = mybir.dt.bfloat16
    f8e3 = mybir.dt.float8e3
    T = T_big + T_small
    ST_W = T * 128
    DB_big = -(-T_big // 32)             # double-bank fills (1024 f32 cols)
    DB = DB_big + -(-T_small // 16)

    nc = bacc.Bacc("TRN2", target_bir_lowering=False, debug=False,
                   num_devices=NCORES)
    data_dram = nc.dram_tensor(
        "data", [128, ST_W + 4096], bf16, kind="ExternalInput").ap()
    MV_W = (2 * T_big + 4 * T_small) * 64
    mv_dram = nc.dram_tensor("mv8", [128, MV_W], f8e3,
                             kind="ExternalInput").ap()
    out_dram = nc.dram_tensor("out", [128, 2 * U], bf16,
                              kind="ExternalOutput").ap()

    CH = 16                              # DMA / matmul chunk size (tiles)

    with tile.TileContext(nc) as tc, ExitStack() as ctx:
        big = ctx.enter_context(tc.tile_pool(name="big", bufs=1))
        cpool = ctx.enter_context(tc.tile_pool(name="cps", bufs=3,
                                               space="PSUM"))
        opool = ctx.enter_context(tc.tile_pool(name="ops", bufs=1,
                                               space="PSUM"))

        st_sb = big.tile([128, ST_W], bf16, tag="st")
        # full-height fp8 mv image: all slot-separation zeros shipped from
        # the host, so every DMA in the kernel is dependency-free and the
        # scheduler cannot create queue head-blocks.
        mv_sb = big.tile([128, MV_W], f8e3, tag="mv")
        w2_sb = big.tile([128, 4096], bf16, tag="w2")
        c_spread = big.tile([128, DB * 1024], bf16, tag="csp")
        out_sb = big.tile([128, 2 * U], bf16, tag="outsb")
        wu_sb = big.tile([128, 64], bf16, tag="wu")

        # PE warm-up with full-128-row matmuls (1-row matmuls do NOT
        # register as PE-busy for the HAM clock gate): ~4.5us of sustained
        # activity un-throttles the PE clock 1.2 -> 2.4 GHz.
        nc.vector.memset(wu_sb[:], 0.25)
        wups = cpool.tile([128, 1024], f32, tag="cps", name="wups")
        for _ in range(90):
            nc.tensor.matmul(wups[0:64, 0:64], wu_sb[:, 0:64],
                             wu_sb[:, 0:64], start=True, stop=True)

        # w2 rides the otherwise-idle SWDGE queue, off the two HWDGE queues
        nc.gpsimd.dma_start(w2_sb[:], data_dram[:, ST_W:])

        # all DMAs dep-free, paired per chunk across the two HWDGE queues
        q = [nc.sync, nc.scalar]
        nbig_ch = -(-T_big // CH)
        nsml_ch = -(-T_small // CH)
        for k in range(nbig_ch):
            b0, b1 = k * CH, min((k + 1) * CH, T_big)
            q[k % 2].dma_start(st_sb[:, b0 * 128:b1 * 128],
                               data_dram[:, b0 * 128:b1 * 128])
            for j in range(2):
                o = j * T_big * 64
                q[(k + 1) % 2].dma_start(
                    mv_sb[:, o + b0 * 64:o + b1 * 64],
                    mv_dram[:, o + b0 * 64:o + b1 * 64])
        MVS0 = 2 * T_big * 64
        for k in range(nsml_ch):
            s0, s1 = k * CH, min((k + 1) * CH, T_small)
            kk = k + nbig_ch
            q[kk % 2].dma_start(
                st_sb[:, (T_big + s0) * 128:(T_big + s1) * 128],
                data_dram[:, (T_big + s0) * 128:(T_big + s1) * 128])
            for j in range(4):
                o = MVS0 + j * T_small * 64
                q[(kk + 1) % 2].dma_start(
                    mv_sb[:, o + s0 * 64:o + s1 * 64],
                    mv_dram[:, o + s0 * 64:o + s1 * 64])

        # C stage; whole-double-bank drains (1024 f32 cols) amortize the
        # per-op overhead; alternate DVE/ACT 4:3 (DVE is faster)
        di = [0]

        def drain(ps, db, w=1024):
            eng = nc.vector.tensor_copy if di[0] % 2 == 0 else nc.scalar.copy
            di[0] += 1
            eng(c_spread[:, db * 1024:db * 1024 + w], ps[:, 0:w])

        # zero the pad tails of partial double-banks (never matmul-written)
        if T_big * 32 < DB_big * 1024:
            nc.vector.memset(c_spread[:, T_big * 32:DB_big * 1024], 0.0)
        if DB_big * 1024 + T_small * 64 < DB * 1024:
            nc.vector.memset(
                c_spread[:, DB_big * 1024 + T_small * 64:DB * 1024], 0.0)

        # W stage: po_g[m, u] += sum_f W[f, m*16+h] * C[u, g, f, h]
        # 4 strips on 4 distinct row groups; strips (0,1)->poA, (2,3)->poB
        # packed into col groups 0/64 of the same PSUM bank.  Split into
        # two u-halves: half 1 is issued mid-C-stage (its banks are
        # already drained) so it runs while the PE would otherwise wait
        # on DMA; only half 2 remains as tail work.
        poA_full = opool.tile([128, 512], f32, tag="poA", name="poA")
        poB_full = opool.tile([128, 512], f32, tag="poB", name="poB")
        poA, poB = poA_full[:, 0:U], poB_full[:, 0:U]
        po2 = [poA, poB]
        BU = 2 * T_big                   # used big u-slots
        S0 = DB_big * 64                 # first small u-slot
        SU = S0 + 4 * T_small            # end of used small u-slots
        # zero the never-written pad u-slots so the out copies are clean
        for pf in (poA_full, poB_full):
            if BU < S0:
                nc.vector.memset(pf[:, BU:S0], 0.0)
            if SU < U:
                nc.vector.memset(pf[:, SU:U], 0.0)

        def w_stage(u0, u1):
            for h in range(HID):
                for s in range(2):   # two full-K=128 matmuls per phase
                    nc.tensor.matmul(
                        po2[s][:, u0:u1],
                        w2_sb[:, 2048 * s + 128 * h:2048 * s + 128 * h + 128],
                        c_spread[:, u0 * 16 + h:u1 * 16:16],
                        start=(h == 0), stop=(h == HID - 1),
                        skip_group_check=True)

        mv_big = mv_sb[:, 0:MVS0].rearrange("p (r c) -> p r c", r=2)
        mv_small = mv_sb[:, MVS0:].rearrange("p (r c) -> p r c", r=4)

        ps = None

        # Each tile runs as 4 concurrent col-group matmuls (one per ef
        # class, tile_position=(0, 32g)): class g writes only partitions
        # [32g, 32g+32), so the four classes stack into the SAME psum
        # columns at different partition bands -- 4x denser PSUM and 4x
        # less drain traffic than the fused [128, N] matmul, with the
        # same stationary/moving data.
        def mm_big_tile(t):
            nonlocal ps
            if t % 32 == 0:
                ps = cpool.tile([128, 1024], f32, tag="cps")
            tr = t
            c0 = (t % 32) * 32
            for g in range(4):
                nc.tensor.matmul(
                    ps[32 * g:32 * g + 32, c0:c0 + 32],
                    st_sb[:, t * 128 + 32 * g:t * 128 + 32 * g + 32],
                    mv_big[:, :, tr * 64 + 16 * g:tr * 64 + 16 * g + 16],
                    start=True, stop=True, skip_group_check=True,
                    tile_position=(0, 32 * g))
            if t % 32 == 31 or t == T_big - 1:
                drain(ps, t // 32,
                      min(1024, (T_big - 32 * (t // 32)) * 32))

        def mm_small_tile(ts):
            nonlocal ps
            if ts % 16 == 0:
                ps = cpool.tile([128, 1024], f32, tag="cps")
            tr = ts
            c0 = (ts % 16) * 64
            for g in range(4):
                nc.tensor.matmul(
                    ps[32 * g:32 * g + 32, c0:c0 + 64],
                    st_sb[:, (T_big + ts) * 128 + 32 * g:
                          (T_big + ts) * 128 + 32 * g + 32],
                    mv_small[:, :, tr * 64 + 16 * g:tr * 64 + 16 * g + 16],
                    start=True, stop=True, skip_group_check=True,
                    tile_position=(0, 32 * g))
            if ts % 16 == 15 or ts == T_small - 1:
                sdb = ts // 16
                drain(ps, DB_big + sdb,
                      min(1024, (T_small - 16 * sdb) * 64))
                if ts != T_small - 1:
                    w_stage(S0 + sdb * 64,
                            min(S0 + (sdb + 1) * 64, SU))

        for t in range(T_big):
            mm_big_tile(t)
        w_stage(0, BU)           # W big-half (pads skipped)
        last_sdb = (T_small - 1) // 16
        for ts in range(T_small):
            mm_small_tile(ts)
        w_stage(S0 + last_sdb * 64, SU)   # only the last double as tail
        nc.vector.tensor_copy(out_sb[:, 0:U], poA[:])
        nc.scalar.copy(out_sb[:, U:2 * U], poB[:])
        nc.sync.dma_start(out_dram[:], out_sb[:])
    nc.compile()
    return nc


def _assemble(outs, segs_per_core, nbig_per_core, T_big, U):
    out = np.zeros((N_NODES, MSG), dtype=np.float32)
    mrow = np.arange(MSG)[None, :]
    for c in range(NCORES):
        segs = segs_per_core[c]
        nbig = nbig_per_core[c]
        P = len(segs)
        if P == 0:
            continue
        po_sb = outs[c].astype(np.float32)           # [128, 2U]
        i = np.arange(P)
        DB_big = -(-T_big // 32)
        p = np.where(i < nbig, i, 256 * DB_big + (i - nbig))  # grid position
        u, g = p // 4, p % 4
        part = 64 * (g % 2)[:, None] + mrow          # [P, 64]
        col = ((g // 2) * U + u)[:, None]
        pos_rows = po_sb[part, col]                  # [P, 64]
        nodes = np.fromiter((segs[k][0] for k in range(P)), dtype=np.int64,
                            count=P)
        np.add.at(out, nodes, pos_rows)
    return out


def kernel(node_features, edge_features, edge_sources, edge_targets,
           hidden, initial, W, b):
    from concourse.bass_utils import run_bass_kernel_spmd

    edge_targets = np.asarray(edge_targets)
    edge_sources = np.asarray(edge_sources)
    edge_features = np.asarray(edge_features, dtype=np.float32)
    hidden = np.asarray(hidden, dtype=np.float32)
    W = np.asarray(W, dtype=np.float32)
    b = np.asarray(b, dtype=np.float32)

    key = edge_targets.tobytes()
    if key in _CACHE:
        layout, nc = _CACHE[key]
    else:
        layout = _build_layout(edge_targets)
        segs_per_core, nbig_per_core, T_big, T_small, U = layout
        nc = _build_program(T_big, T_small, U)
        _CACHE[key] = (layout, nc)
    segs_per_core, nbig_per_core, T_big, T_small, U = layout

    w2 = _build_w2(W)
    in_maps = []
    for c in range(NCORES):
        data, mv8 = _pack_core(segs_per_core[c], nbig_per_core[c], T_big,
                               T_small, w2, edge_features, edge_sources,
                               hidden)
        in_maps.append({"data": data, "mv8": mv8})

    res = run_bass_kernel_spmd(nc, in_maps, list(range(NCORES)))
    outs = [res.results[c]["out"] for c in range(NCORES)]
    out = _assemble(outs, segs_per_core, nbig_per_core, T_big, U)

    if np.any(b):
        # bias term: out[n] += (sum_{e->n} hidden[src e]) @ Br,
        # Br[h, m] = b[m*16+h].  (b is all-zero for this problem.)
        Br = b.reshape(MSG, HID).T.astype(np.float32)
        acc = np.zeros((N_NODES, HID), dtype=np.float32)
        np.add.at(acc, edge_targets, hidden[edge_sources])
        out += acc @ Br
    return out

